# revision 6
# baseline (speedup 1.0000x reference)
"""ALIGNN layer (edge MLP + mean-aggregation + node MLP) on 8 TRN2 NeuronCores.

Sharding: edges are partitioned by destination-node ownership — core k owns
nodes [k*2500, (k+1)*2500) and receives every edge whose dst falls in its
range, grouped by 125-node chunk so the per-node segment-sum is computed
locally with one-hot selection matmuls (no collectives needed).

Math decomposition (per edge e with endpoints s=src[e], d=dst[e]):
    z[e]  = edge[e] @ W1e[:128] + P1[s] + P2[d]        (+ b1e folded into silu bias)
    h[e]  = silu(z[e]);  edge_out[e] = edge[e] + b2e + h[e] @ W2e
where P1 = node_feat @ W1e[128:256], P2 = node_feat @ W1e[256:384] are
per-node projection tables computed on-device once. P1[src] is gathered per
edge (dma_gather, 4 SWDGE queues); P2[dst] is reconstructed without any
gather as P2_chunk.T @ one-hot(dst) since dst is chunk-local by construction.

All activations live feature-on-partition ("transposed") so the per-feature
biases become per-partition ACT biases; host supplies edge features / node
features pre-transposed plus the one-hot selection tiles, and un-transposes
the outputs.
"""
import os
import sys
import types
import numpy as np
import ml_dtypes
from contextlib import ExitStack


def _install_ntff_hook_shim():
    """This image's `antenv` lacks `axon_hooks`, so bass_utils' trace=True path
    (BASS_TRACE=1) crashes on import. Provide the two-function shim and register
    the ctypes-based NTFF hook from trn_boot so neuron-profile works."""
    try:
        import antenv
        if "antenv.axon_hooks" not in sys.modules:
            mod = types.ModuleType("antenv.axon_hooks")
            _h = [None]
            mod.set_axon_ntff_profile_hook = lambda h: _h.__setitem__(0, h)
            mod.get_axon_ntff_profile_hook = lambda: _h[0]
            sys.modules["antenv.axon_hooks"] = mod
            antenv.axon_hooks = mod
        from trn_agent_boot import trn_boot
        hook = trn_boot._ntff_profile_via_ctypes("/opt/axon/libaxon_pjrt.so")
        if hook is not None:
            sys.modules["antenv.axon_hooks"].set_axon_ntff_profile_hook(hook)
    except Exception:
        pass


_install_ntff_hook_shim()

from concourse import bacc, mybir
from concourse.tile import TileContext
from concourse.bass_utils import run_bass_kernel_spmd

BF16 = ml_dtypes.bfloat16
F32 = np.float32
dt = mybir.dt
AF = mybir.ActivationFunctionType
ALU = mybir.AluOpType

N_NODES = 20000
N_EDGES = 640000
D = 128
NCORES = 8
NPC = N_NODES // NCORES        # 2500 nodes per core
CH = 125                       # nodes per chunk
NCH = NPC // CH                # 20 chunks per core
TILE = 128
ET = 512                       # edges per pipeline group (4 tiles)
GW = 4 * CH + 4 * TILE         # sel+selT columns per group (500 + 512)
SW = ET + GW                   # full per-group stream width (edge + sel + selT)
NF_PAD = 20480                 # node table rows padded to 160*128

LAST_RESULTS = None            # BassKernelResults of the most recent run


def _build(T):
    """Build the SPMD Bass program. T[c] = tiles per chunk (same all cores)."""
    total_tiles = int(np.sum(T))
    E_pad = total_tiles * TILE
    n_groups = E_pad // ET
    off = np.concatenate([[0], np.cumsum(T)])
    tile_chunk = np.repeat(np.arange(NCH), T)
    first = off[:-1]
    last = off[1:] - 1

    nc = bacc.Bacc("TRN2", target_bir_lowering=False, debug=False,
                   num_swdge_queues=4)

    p_stream = nc.declare_dram_parameter("stream_all", [TILE, n_groups * SW], dt.bfloat16, False)
    p_srcw = nc.declare_dram_parameter("src_w", [TILE, E_pad // 16], dt.int16, False)
    p_ident = nc.declare_dram_parameter("ident", [TILE, TILE], dt.bfloat16, False)
    p_invc = nc.declare_dram_parameter("inv_cnt", [TILE, NCH], dt.float32, False)
    p_nft = nc.declare_dram_parameter("nft_full", [TILE, NF_PAD], dt.bfloat16, False)
    p_nfto = nc.declare_dram_parameter("nft_own", [TILE, NPC], dt.bfloat16, False)
    wnames = ["w1a", "w1b", "w1c", "w2e", "w1na", "w1nb", "w2n"]
    p_w = {n: nc.declare_dram_parameter(n, [TILE, TILE], dt.bfloat16, False) for n in wnames}
    bnames = ["b1e", "b1n", "b2n"]
    p_b = {n: nc.declare_dram_parameter(n, [TILE, 1], dt.float32, False) for n in bnames}
    out_e = nc.declare_dram_parameter("edge_out", [TILE, E_pad], dt.bfloat16, True)
    out_n = nc.declare_dram_parameter("node_out", [TILE, NPC], dt.bfloat16, True)

    P2o = nc.dram_tensor("p2o", [NPC, D], dt.bfloat16)

    with TileContext(nc) as tc, ExitStack() as ctx:
        cp = ctx.enter_context(tc.tile_pool(name="const", bufs=1))
        wp = ctx.enter_context(tc.tile_pool(name="work", bufs=3))
        pp = ctx.enter_context(tc.tile_pool(name="psum", bufs=2, space="PSUM"))

        def cload(name, param, shape, dtype):
            t = cp.tile(shape, dtype, name=name)
            nc.sync.dma_start(out=t[:], in_=param[:])
            return t

        ident = cload("identc", p_ident, [TILE, TILE], dt.bfloat16)
        srcw = cload("srcwc", p_srcw, [TILE, E_pad // 16], dt.int16)
        invc = cload("invcc", p_invc, [TILE, NCH], dt.float32)
        nfto = cload("nftoc", p_nfto, [TILE, NPC], dt.bfloat16)
        w = {n: cload(n + "c", p_w[n], [TILE, TILE], dt.bfloat16) for n in wnames}
        b = {n: cload(n + "c", p_b[n], [TILE, 1], dt.float32) for n in bnames}
        aggbar = cp.tile([TILE, NCH * TILE], dt.bfloat16, name="aggbar")
        nout = cp.tile([TILE, NPC], dt.bfloat16, name="nout")
        p1sb = cp.tile([TILE, (NF_PAD // TILE) * TILE], dt.bfloat16, name="p1sb")

        # ---- Phase 0: node projection tables P1 = nf @ W1e[128:256], P2 = nf @ W1e[256:384]
        for blk in range(NF_PAD // ET):
            nft = wp.tile([TILE, ET], dt.bfloat16, tag="nft", bufs=3, name=f"nft{blk}")
            nc.scalar.dma_start(out=nft[:], in_=p_nft[:, blk * ET:(blk + 1) * ET])
            ps = pp.tile([TILE, ET], dt.float32, tag="hps", bufs=2, name=f"ps0_{blk}")
            for j in range(4):
                nc.tensor.matmul(out=ps[:, j * TILE:(j + 1) * TILE],
                                 lhsT=nft[:, j * TILE:(j + 1) * TILE],
                                 rhs=w["w1b"][:], start=True, stop=True)
            if blk % 2 == 0:
                nc.vector.tensor_copy(out=p1sb[:, blk * ET:(blk + 1) * ET], in_=ps[:])
            else:
                nc.scalar.copy(out=p1sb[:, blk * ET:(blk + 1) * ET], in_=ps[:])

        # ---- Phase 0b: per-core own-node projections P2o = nft_own @ W1e[256:384]
        for c in range(NCH):
            ps2 = pp.tile([CH, TILE], dt.float32, tag="agg", bufs=2, name=f"ps2_{c}")
            nc.tensor.matmul(out=ps2[:], lhsT=nfto[:, c * CH:(c + 1) * CH],
                             rhs=w["w1c"][:], start=True, stop=True)
            st2 = wp.tile([CH, TILE], dt.bfloat16, tag="st2", bufs=2, name=f"st2_{c}")
            if c % 2 == 0:
                nc.vector.tensor_copy(out=st2[:], in_=ps2[:])
            else:
                nc.scalar.copy(out=st2[:], in_=ps2[:])
            nc.sync.dma_start(out=P2o[c * CH:(c + 1) * CH, :], in_=st2[:])

        # ---- Phase 1: edge pipeline
        agg_tiles = [None] * NCH
        p2c_tiles = [None] * NCH
        for g in range(n_groups):
            sl_ = slice(g * ET, (g + 1) * ET)
            stg = wp.tile([TILE, SW], dt.bfloat16, tag="stg", bufs=4, name=f"stg{g}")
            nc.sync.dma_start(out=stg[:], in_=p_stream[:, g * SW:(g + 1) * SW])
            ed = stg[:, :ET]
            selg = stg[:, ET:]
            s1 = wp.tile([TILE, ET], dt.bfloat16, tag="s1", bufs=6, name=f"s1_{g}")
            nc.gpsimd.dma_gather(
                out_ap=s1[:].rearrange("p (o e) -> p o e", o=1), in_ap=p1sb[:],
                idxs_ap=srcw[:, g * 32:(g + 1) * 32], num_idxs=ET, num_idxs_reg=ET,
                elem_size=D, transpose=True, queue_num=(g % 4),
                sbuf_tokens_per_rank=TILE, sbuf_free_dim_per_rank=2 * D)

            # chunk-contiguous segments inside the group, and P2 chunk row loads
            segs = []
            for j in range(4):
                c = int(tile_chunk[4 * g + j])
                if segs and segs[-1][0] == c:
                    segs[-1][2] = j
                else:
                    segs.append([c, j, j])
                if 4 * g + j == first[c]:
                    p2c_tiles[c] = wp.tile([CH, TILE], dt.bfloat16, tag="p2c", bufs=2,
                                           name=f"p2c{c}")
                    nc.scalar.dma_start(out=p2c_tiles[c][:],
                                        in_=P2o[c * CH:(c + 1) * CH, :])

            hps = pp.tile([TILE, ET], dt.float32, tag="hps", bufs=2, name=f"hps{g}")
            nc.tensor.matmul(out=hps[:], lhsT=w["w1a"][:], rhs=ed, start=True, stop=False)
            nc.tensor.matmul(out=hps[:], lhsT=ident[:], rhs=s1[:], start=False, stop=False)
            for i, (c, j0, j1) in enumerate(segs):
                nc.tensor.matmul(
                    out=hps[:, j0 * TILE:(j1 + 1) * TILE],
                    lhsT=p2c_tiles[c][:],
                    rhs=selg[:CH, 4 * CH + j0 * TILE: 4 * CH + (j1 + 1) * TILE],
                    start=False, stop=(i == len(segs) - 1), skip_group_check=True)

            ht = wp.tile([TILE, ET], dt.bfloat16, tag="ht", bufs=4, name=f"ht{g}")
            nc.scalar.activation(out=ht[:], in_=hps[:], func=AF.Silu, bias=b["b1e"][:, :1])

            dps = pp.tile([TILE, ET], dt.float32, tag="dps", bufs=2, name=f"dps{g}")
            nc.tensor.matmul(out=dps[:], lhsT=w["w2e"][:], rhs=ht[:], start=True, stop=True)
            eo = wp.tile([TILE, ET], dt.bfloat16, tag="eo", bufs=4, name=f"eo{g}")
            nc.vector.tensor_tensor(out=eo[:], in0=dps[:], in1=ed, op=ALU.add)
            nc.scalar.dma_start(out=out_e[:, sl_], in_=eo[:])

            for j in range(4):
                t = 4 * g + j
                c = int(tile_chunk[t])
                tp = pp.tile([TILE, TILE], dt.bfloat16, tag="tp", bufs=2, name=f"tp{t}")
                nc.tensor.transpose(out=tp[:], in_=eo[:, j * TILE:(j + 1) * TILE],
                                    identity=ident[:])
                rt = wp.tile([TILE, TILE], dt.bfloat16, tag="rt", bufs=4, name=f"rt{t}")
                nc.vector.tensor_copy(out=rt[:], in_=tp[:])
                if t == first[c]:
                    agg_tiles[c] = pp.tile([CH, TILE], dt.float32, tag="agg", bufs=2,
                                           name=f"agg{c}")
                nc.tensor.matmul(out=agg_tiles[c][:],
                                 lhsT=selg[:, j * CH:(j + 1) * CH], rhs=rt[:],
                                 start=(t == first[c]), stop=(t == last[c]),
                                 skip_group_check=True)
                if t == last[c]:
                    nc.vector.tensor_scalar(out=aggbar[:CH, c * TILE:(c + 1) * TILE],
                                            in0=agg_tiles[c][:],
                                            scalar1=invc[:CH, c:c + 1], scalar2=None,
                                            op0=ALU.mult)

        # ---- Phase 2: node MLP per chunk
        for c in range(NCH):
            tp2 = pp.tile([TILE, TILE], dt.bfloat16, tag="tp", bufs=2, name=f"tp2_{c}")
            nc.tensor.transpose(out=tp2[:, :CH], in_=aggbar[:CH, c * TILE:(c + 1) * TILE],
                                identity=ident[:CH, :CH])
            abt = wp.tile([TILE, CH], dt.bfloat16, tag="abt", bufs=2, name=f"abt{c}")
            nc.vector.tensor_copy(out=abt[:], in_=tp2[:, :CH])
            nsl = slice(c * CH, (c + 1) * CH)
            gps = pp.tile([TILE, CH], dt.float32, tag="agg", bufs=2, name=f"gps{c}")
            nc.tensor.matmul(out=gps[:], lhsT=w["w1na"][:], rhs=nfto[:, nsl],
                             start=True, stop=False)
            nc.tensor.matmul(out=gps[:], lhsT=w["w1nb"][:], rhs=abt[:],
                             start=False, stop=True)
            gt = wp.tile([TILE, CH], dt.bfloat16, tag="gt", bufs=2, name=f"gt{c}")
            nc.scalar.activation(out=gt[:], in_=gps[:], func=AF.Silu, bias=b["b1n"][:, :1])
            ops = pp.tile([TILE, CH], dt.float32, tag="tp", bufs=2, name=f"ops{c}")
            nc.tensor.matmul(out=ops[:], lhsT=w["w2n"][:], rhs=gt[:], start=True, stop=True)
            t3 = wp.tile([TILE, CH], dt.bfloat16, tag="t3", bufs=2, name=f"t3_{c}")
            nc.scalar.activation(out=t3[:], in_=ops[:], func=AF.Identity, bias=b["b2n"][:, :1])
            nc.vector.tensor_tensor(out=nout[:, nsl], in0=t3[:], in1=nfto[:, nsl], op=ALU.add)
        nc.sync.dma_start(out=out_n[:], in_=nout[:])

    nc.compile()
    return nc


def kernel(**inputs):
    global LAST_RESULTS
    node_feat = np.ascontiguousarray(np.asarray(inputs["node_feat"], F32))
    edge_feat = np.ascontiguousarray(np.asarray(inputs["edge_feat"], F32))
    edge_index = np.asarray(inputs["edge_index"])
    W1e = np.asarray(inputs["W1e"], F32)
    b1e = np.asarray(inputs["b1e"], F32)
    W2e = np.asarray(inputs["W2e"], F32)
    b2e = np.asarray(inputs["b2e"], F32)
    W1n = np.asarray(inputs["W1n"], F32)
    b1n = np.asarray(inputs["b1n"], F32)
    W2n = np.asarray(inputs["W2n"], F32)
    b2n = np.asarray(inputs["b2n"], F32)

    src = edge_index[0].astype(np.int64)
    dst = edge_index[1].astype(np.int64)

    # ---- host-side distribution: bucket edges by (owner core, 125-node chunk)
    core = dst // NPC
    loc = dst - core * NPC
    chunk = loc // CH
    lid = (loc - chunk * CH).astype(np.int64)
    key = core * NCH + chunk
    order = np.argsort(key, kind="stable")
    counts = np.bincount(key, minlength=NCORES * NCH).reshape(NCORES, NCH)
    T = np.maximum(1, -(-counts // TILE)).max(axis=0)        # tiles per chunk
    T[-1] += (-T.sum()) % (ET // TILE)                       # total multiple of 4
    total_tiles = int(T.sum())
    E_pad = total_tiles * TILE
    n_groups = E_pad // ET
    off = np.concatenate([[0], np.cumsum(T)])
    key_starts = np.concatenate([[0], np.cumsum(counts.reshape(-1))])

    eids_all = np.full((NCORES, E_pad), -1, np.int64)
    for k in range(NCORES):
        for c in range(NCH):
            cnt = counts[k, c]
            ids = order[key_starts[k * NCH + c]: key_starts[k * NCH + c] + cnt]
            eids_all[k, off[c] * TILE: off[c] * TILE + cnt] = ids

    nc = _build(T)

    nfT = np.zeros((TILE, NF_PAD), BF16)
    nfT[:, :N_NODES] = node_feat.T.astype(BF16)
    ident_t = np.eye(TILE, dtype=F32).astype(BF16)
    node_cnt = np.bincount(dst, minlength=N_NODES).astype(F32)
    invc_full = (1.0 / np.maximum(node_cnt, 1.0)).reshape(NCORES, NCH, CH)

    wmats = {"w1a": W1e[:D], "w1b": W1e[D:2 * D], "w1c": W1e[2 * D:],
             "w2e": W2e, "w1na": W1n[:D], "w1nb": W1n[D:], "w2n": W2n}
    b1e_adj = b1e - b2e @ W1e[:D]          # b2 pre-added to edges passes through W1a
    bvecs = {"b1e": b1e_adj, "b1n": b1n, "b2n": b2n}

    def wrap16(vals):
        # index i -> [i % 16, i // 16], replicated across the 8 Q7 groups
        return np.ascontiguousarray(
            np.tile(vals.reshape(-1, 16).T, (8, 1))).astype(np.int16)

    nrange = np.arange(CH, dtype=np.int64)

    in_maps = []
    for k in range(NCORES):
        eids = eids_all[k]
        valid = eids >= 0
        ev = np.zeros((E_pad, D), F32)
        ev[valid] = edge_feat[eids[valid]] + b2e[None, :]
        sw = np.zeros(E_pad, np.int64)
        sw[valid] = src[eids[valid]]
        dl = np.full(E_pad, CH + 1, np.int64)     # padding never matches 0..124
        dl[valid] = lid[eids[valid]]

        # one-hot selection tiles: sel [e,n] and its transpose selT [n,e]
        dlt = dl.reshape(total_tiles, TILE)                     # [t, e]
        sel_t = (dlt[:, :, None] == nrange[None, None, :])      # [t, e, n]
        selT_t = np.zeros((total_tiles, TILE, TILE), np.bool_)  # [t, n(128), e]
        selT_t[:, :CH, :] = sel_t.transpose(0, 2, 1)
        sel_g = sel_t.reshape(n_groups, 4, TILE, CH).transpose(0, 2, 1, 3).reshape(
            n_groups, TILE, 4 * CH)
        selT_g = selT_t.reshape(n_groups, 4, TILE, TILE).transpose(0, 2, 1, 3).reshape(
            n_groups, TILE, 4 * TILE)
        ed_g = ev.T.astype(BF16).reshape(TILE, n_groups, ET).transpose(1, 0, 2)
        stream = np.concatenate([ed_g, sel_g.astype(BF16), selT_g.astype(BF16)],
                                axis=2)                          # [g, 128, SW]
        stream = np.ascontiguousarray(
            stream.transpose(1, 0, 2).reshape(TILE, n_groups * SW))

        invc_t = np.zeros((TILE, NCH), F32)
        invc_t[:CH] = invc_full[k].T
        in_map = {
            "stream_all": stream,
            "src_w": wrap16(sw.astype(np.int16)),
            "ident": ident_t,
            "inv_cnt": invc_t,
            "nft_full": nfT,
            "nft_own": np.ascontiguousarray(node_feat[k * NPC:(k + 1) * NPC].T.astype(BF16)),
            **{n: np.ascontiguousarray(m.astype(BF16)) for n, m in wmats.items()},
            **{n: np.ascontiguousarray(v.reshape(D, 1).astype(F32)) for n, v in bvecs.items()},
        }
        in_maps.append(in_map)

    LAST_RESULTS = run_bass_kernel_spmd(nc, in_maps, core_ids=list(range(NCORES)))
    results = LAST_RESULTS.results

    node_out = np.concatenate(
        [results[k]["node_out"].astype(F32).T for k in range(NCORES)], axis=0)
    edge_out = np.empty((N_EDGES, D), F32)
    for k in range(NCORES):
        eids = eids_all[k]
        valid = eids >= 0
        eo = results[k]["edge_out"].astype(F32).T
        edge_out[eids[valid]] = eo[valid]
    return (node_out, edge_out)


# revision 7
# speedup vs baseline: 1.5214x; 1.5214x over previous
"""ALIGNN layer (edge MLP + mean-aggregation + node MLP) on 8 TRN2 NeuronCores.

Sharding: edges are partitioned by destination-node ownership — core k owns
nodes [k*2500, (k+1)*2500) and receives every edge whose dst falls in its
range, grouped by 125-node chunk so the per-node segment-sum is computed
locally with one-hot selection matmuls (no collectives needed).

Math decomposition (per edge e with endpoints s=src[e], d=dst[e]):
    z[e]  = edge[e] @ W1e[:128] + P1[s] + P2[d]        (+ b1e folded into silu bias)
    h[e]  = silu(z[e]);  edge_out[e] = edge[e] + b2e + h[e] @ W2e
where P1 = node_feat @ W1e[128:256], P2 = node_feat @ W1e[256:384] are
per-node projection tables computed on-device once. P1[src] is gathered per
edge (dma_gather, 4 SWDGE queues); P2[dst] is reconstructed without any
gather as P2_chunk.T @ one-hot(dst) since dst is chunk-local by construction.

All activations live feature-on-partition ("transposed") so the per-feature
biases become per-partition ACT biases; host supplies edge features / node
features pre-transposed plus the one-hot selection tiles, and un-transposes
the outputs.
"""
import os
import sys
import types
import numpy as np
import ml_dtypes
from contextlib import ExitStack


def _install_ntff_hook_shim():
    """This image's `antenv` lacks `axon_hooks`, so bass_utils' trace=True path
    (BASS_TRACE=1) crashes on import. Provide the two-function shim and register
    the ctypes-based NTFF hook from trn_boot so neuron-profile works."""
    try:
        import antenv
        if "antenv.axon_hooks" not in sys.modules:
            mod = types.ModuleType("antenv.axon_hooks")
            _h = [None]
            mod.set_axon_ntff_profile_hook = lambda h: _h.__setitem__(0, h)
            mod.get_axon_ntff_profile_hook = lambda: _h[0]
            sys.modules["antenv.axon_hooks"] = mod
            antenv.axon_hooks = mod
        from trn_agent_boot import trn_boot
        hook = trn_boot._ntff_profile_via_ctypes("/opt/axon/libaxon_pjrt.so")
        if hook is not None:
            sys.modules["antenv.axon_hooks"].set_axon_ntff_profile_hook(hook)
    except Exception:
        pass


_install_ntff_hook_shim()

from concourse import bacc, mybir
from concourse.tile import TileContext
from concourse.bass_utils import run_bass_kernel_spmd

BF16 = ml_dtypes.bfloat16
F32 = np.float32
dt = mybir.dt
AF = mybir.ActivationFunctionType
ALU = mybir.AluOpType

N_NODES = 20000
N_EDGES = 640000
D = 128
NCORES = 8
NPC = N_NODES // NCORES        # 2500 nodes per core
CH = 125                       # nodes per chunk
NCH = NPC // CH                # 20 chunks per core
TILE = 128
ET = 512                       # edges per pipeline group (4 tiles)
GW = 4 * CH + 4 * TILE         # sel+selT columns per group (500 + 512)
SW = ET + GW                   # full per-group stream width (edge + sel + selT)
NF_PAD = 20480                 # node table rows padded to 160*128

LAST_RESULTS = None            # BassKernelResults of the most recent run


def _build(T):
    """Build the SPMD Bass program. T[c] = tiles per chunk (same all cores)."""
    total_tiles = int(np.sum(T))
    E_pad = total_tiles * TILE
    n_groups = E_pad // ET
    off = np.concatenate([[0], np.cumsum(T)])
    tile_chunk = np.repeat(np.arange(NCH), T)
    first = off[:-1]
    last = off[1:] - 1

    nc = bacc.Bacc("TRN2", target_bir_lowering=False, debug=False,
                   num_swdge_queues=4)

    p_stream = nc.declare_dram_parameter("stream_all", [TILE, n_groups * SW], dt.bfloat16, False)
    p_srcw = nc.declare_dram_parameter("src_w", [TILE, E_pad // 16], dt.int16, False)
    p_ident = nc.declare_dram_parameter("ident", [TILE, TILE], dt.bfloat16, False)
    p_invc = nc.declare_dram_parameter("inv_cnt", [TILE, NCH], dt.float32, False)
    p_nft = nc.declare_dram_parameter("nft_full", [TILE, NF_PAD], dt.bfloat16, False)
    p_nfto = nc.declare_dram_parameter("nft_own", [TILE, NPC], dt.bfloat16, False)
    wnames = ["w1a", "w1b", "w1c", "w2e", "w1na", "w1nb", "w2n"]
    p_w = {n: nc.declare_dram_parameter(n, [TILE, TILE], dt.bfloat16, False) for n in wnames}
    bnames = ["b1e", "b1n", "b2n"]
    p_b = {n: nc.declare_dram_parameter(n, [TILE, 1], dt.float32, False) for n in bnames}
    out_e = nc.declare_dram_parameter("edge_out", [TILE, E_pad], dt.bfloat16, True)
    out_n = nc.declare_dram_parameter("node_out", [TILE, NPC], dt.bfloat16, True)

    P1d = nc.dram_tensor("p1d", [NF_PAD, D], dt.bfloat16)
    P2o = nc.dram_tensor("p2o", [NPC, D], dt.bfloat16)

    with TileContext(nc) as tc, ExitStack() as ctx:
        cp = ctx.enter_context(tc.tile_pool(name="const", bufs=1))
        wp = ctx.enter_context(tc.tile_pool(name="work", bufs=3))
        pp = ctx.enter_context(tc.tile_pool(name="psum", bufs=2, space="PSUM"))

        def cload(name, param, shape, dtype):
            t = cp.tile(shape, dtype, name=name)
            nc.sync.dma_start(out=t[:], in_=param[:])
            return t

        ident = cload("identc", p_ident, [TILE, TILE], dt.bfloat16)
        srcw = cload("srcwc", p_srcw, [TILE, E_pad // 16], dt.int16)
        invc = cload("invcc", p_invc, [TILE, NCH], dt.float32)
        nfto = cload("nftoc", p_nfto, [TILE, NPC], dt.bfloat16)
        w = {n: cload(n + "c", p_w[n], [TILE, TILE], dt.bfloat16) for n in wnames}
        b = {n: cload(n + "c", p_b[n], [TILE, 1], dt.float32) for n in bnames}
        aggbar = cp.tile([TILE, NCH * TILE], dt.bfloat16, name="aggbar")
        nout = cp.tile([TILE, NPC], dt.bfloat16, name="nout")

        # ---- Phase 0: node projection tables P1 = nf @ W1e[128:256], P2 = nf @ W1e[256:384]
        for blk in range(NF_PAD // ET):
            nft = wp.tile([TILE, ET], dt.bfloat16, tag="nft", bufs=3, name=f"nft{blk}")
            nc.scalar.dma_start(out=nft[:], in_=p_nft[:, blk * ET:(blk + 1) * ET])
            ps = pp.tile([TILE, ET], dt.float32, tag="hps", bufs=2, name=f"ps0_{blk}")
            for j in range(4):
                nc.tensor.matmul(out=ps[:, j * TILE:(j + 1) * TILE],
                                 lhsT=nft[:, j * TILE:(j + 1) * TILE],
                                 rhs=w["w1b"][:], start=True, stop=True)
            st = wp.tile([TILE, ET], dt.bfloat16, tag="st1", bufs=2, name=f"st1_{blk}")
            if blk % 2 == 0:
                nc.vector.tensor_copy(out=st[:], in_=ps[:])
            else:
                nc.scalar.copy(out=st[:], in_=ps[:])
            dst_ap = P1d[blk * ET:(blk + 1) * ET, :].rearrange("(j p) h -> p j h", p=TILE)
            nc.sync.dma_start(out=dst_ap, in_=st[:].rearrange("p (j h) -> p j h", j=4))

        # ---- Phase 0b: per-core own-node projections P2o = nft_own @ W1e[256:384]
        for c in range(NCH):
            ps2 = pp.tile([CH, TILE], dt.float32, tag="agg", bufs=2, name=f"ps2_{c}")
            nc.tensor.matmul(out=ps2[:], lhsT=nfto[:, c * CH:(c + 1) * CH],
                             rhs=w["w1c"][:], start=True, stop=True)
            st2 = wp.tile([CH, TILE], dt.bfloat16, tag="st2", bufs=2, name=f"st2_{c}")
            if c % 2 == 0:
                nc.vector.tensor_copy(out=st2[:], in_=ps2[:])
            else:
                nc.scalar.copy(out=st2[:], in_=ps2[:])
            nc.sync.dma_start(out=P2o[c * CH:(c + 1) * CH, :], in_=st2[:])

        # ---- Phase 1: edge pipeline
        agg_tiles = [None] * NCH
        p2c_tiles = [None] * NCH
        for g in range(n_groups):
            sl_ = slice(g * ET, (g + 1) * ET)
            stg = wp.tile([TILE, SW], dt.bfloat16, tag="stg", bufs=4, name=f"stg{g}")
            nc.sync.dma_start(out=stg[:], in_=p_stream[:, g * SW:(g + 1) * SW])
            ed = stg[:, :ET]
            selg = stg[:, ET:]
            s1 = wp.tile([TILE, ET], dt.bfloat16, tag="s1", bufs=6, name=f"s1_{g}")
            nc.gpsimd.dma_gather(
                out_ap=s1[:].rearrange("p (o e) -> p o e", o=1), in_ap=P1d[:],
                idxs_ap=srcw[:, g * 32:(g + 1) * 32], num_idxs=ET, num_idxs_reg=ET,
                elem_size=D, transpose=True, queue_num=(g % 4))

            # chunk-contiguous segments inside the group, and P2 chunk row loads
            segs = []
            for j in range(4):
                c = int(tile_chunk[4 * g + j])
                if segs and segs[-1][0] == c:
                    segs[-1][2] = j
                else:
                    segs.append([c, j, j])
                if 4 * g + j == first[c]:
                    p2c_tiles[c] = wp.tile([CH, TILE], dt.bfloat16, tag="p2c", bufs=2,
                                           name=f"p2c{c}")
                    nc.scalar.dma_start(out=p2c_tiles[c][:],
                                        in_=P2o[c * CH:(c + 1) * CH, :])

            hps = pp.tile([TILE, ET], dt.float32, tag="hps", bufs=2, name=f"hps{g}")
            nc.tensor.matmul(out=hps[:], lhsT=w["w1a"][:], rhs=ed, start=True, stop=False)
            nc.tensor.matmul(out=hps[:], lhsT=ident[:], rhs=s1[:], start=False, stop=False)
            for i, (c, j0, j1) in enumerate(segs):
                nc.tensor.matmul(
                    out=hps[:, j0 * TILE:(j1 + 1) * TILE],
                    lhsT=p2c_tiles[c][:],
                    rhs=selg[:CH, 4 * CH + j0 * TILE: 4 * CH + (j1 + 1) * TILE],
                    start=False, stop=(i == len(segs) - 1), skip_group_check=True)

            ht = wp.tile([TILE, ET], dt.bfloat16, tag="ht", bufs=4, name=f"ht{g}")
            nc.scalar.activation(out=ht[:], in_=hps[:], func=AF.Silu, bias=b["b1e"][:, :1])

            dps = pp.tile([TILE, ET], dt.float32, tag="dps", bufs=2, name=f"dps{g}")
            nc.tensor.matmul(out=dps[:], lhsT=w["w2e"][:], rhs=ht[:], start=True, stop=True)
            eo = wp.tile([TILE, ET], dt.bfloat16, tag="eo", bufs=4, name=f"eo{g}")
            nc.vector.tensor_tensor(out=eo[:], in0=dps[:], in1=ed, op=ALU.add)
            nc.scalar.dma_start(out=out_e[:, sl_], in_=eo[:])

            for j in range(4):
                t = 4 * g + j
                c = int(tile_chunk[t])
                tp = pp.tile([TILE, TILE], dt.bfloat16, tag="tp", bufs=2, name=f"tp{t}")
                nc.tensor.transpose(out=tp[:], in_=eo[:, j * TILE:(j + 1) * TILE],
                                    identity=ident[:])
                rt = wp.tile([TILE, TILE], dt.bfloat16, tag="rt", bufs=4, name=f"rt{t}")
                nc.vector.tensor_copy(out=rt[:], in_=tp[:])
                if t == first[c]:
                    agg_tiles[c] = pp.tile([CH, TILE], dt.float32, tag="agg", bufs=2,
                                           name=f"agg{c}")
                nc.tensor.matmul(out=agg_tiles[c][:],
                                 lhsT=selg[:, j * CH:(j + 1) * CH], rhs=rt[:],
                                 start=(t == first[c]), stop=(t == last[c]),
                                 skip_group_check=True)
                if t == last[c]:
                    nc.vector.tensor_scalar(out=aggbar[:CH, c * TILE:(c + 1) * TILE],
                                            in0=agg_tiles[c][:],
                                            scalar1=invc[:CH, c:c + 1], scalar2=None,
                                            op0=ALU.mult)

        # ---- Phase 2: node MLP per chunk
        for c in range(NCH):
            tp2 = pp.tile([TILE, TILE], dt.bfloat16, tag="tp", bufs=2, name=f"tp2_{c}")
            nc.tensor.transpose(out=tp2[:, :CH], in_=aggbar[:CH, c * TILE:(c + 1) * TILE],
                                identity=ident[:CH, :CH])
            abt = wp.tile([TILE, CH], dt.bfloat16, tag="abt", bufs=2, name=f"abt{c}")
            nc.vector.tensor_copy(out=abt[:], in_=tp2[:, :CH])
            nsl = slice(c * CH, (c + 1) * CH)
            gps = pp.tile([TILE, CH], dt.float32, tag="agg", bufs=2, name=f"gps{c}")
            nc.tensor.matmul(out=gps[:], lhsT=w["w1na"][:], rhs=nfto[:, nsl],
                             start=True, stop=False)
            nc.tensor.matmul(out=gps[:], lhsT=w["w1nb"][:], rhs=abt[:],
                             start=False, stop=True)
            gt = wp.tile([TILE, CH], dt.bfloat16, tag="gt", bufs=2, name=f"gt{c}")
            nc.scalar.activation(out=gt[:], in_=gps[:], func=AF.Silu, bias=b["b1n"][:, :1])
            ops = pp.tile([TILE, CH], dt.float32, tag="tp", bufs=2, name=f"ops{c}")
            nc.tensor.matmul(out=ops[:], lhsT=w["w2n"][:], rhs=gt[:], start=True, stop=True)
            t3 = wp.tile([TILE, CH], dt.bfloat16, tag="t3", bufs=2, name=f"t3_{c}")
            nc.scalar.activation(out=t3[:], in_=ops[:], func=AF.Identity, bias=b["b2n"][:, :1])
            nc.vector.tensor_tensor(out=nout[:, nsl], in0=t3[:], in1=nfto[:, nsl], op=ALU.add)
        nc.sync.dma_start(out=out_n[:], in_=nout[:])

    nc.compile()
    return nc


def kernel(**inputs):
    global LAST_RESULTS
    node_feat = np.ascontiguousarray(np.asarray(inputs["node_feat"], F32))
    edge_feat = np.ascontiguousarray(np.asarray(inputs["edge_feat"], F32))
    edge_index = np.asarray(inputs["edge_index"])
    W1e = np.asarray(inputs["W1e"], F32)
    b1e = np.asarray(inputs["b1e"], F32)
    W2e = np.asarray(inputs["W2e"], F32)
    b2e = np.asarray(inputs["b2e"], F32)
    W1n = np.asarray(inputs["W1n"], F32)
    b1n = np.asarray(inputs["b1n"], F32)
    W2n = np.asarray(inputs["W2n"], F32)
    b2n = np.asarray(inputs["b2n"], F32)

    src = edge_index[0].astype(np.int64)
    dst = edge_index[1].astype(np.int64)

    # ---- host-side distribution: bucket edges by (owner core, 125-node chunk)
    core = dst // NPC
    loc = dst - core * NPC
    chunk = loc // CH
    lid = (loc - chunk * CH).astype(np.int64)
    key = core * NCH + chunk
    order = np.argsort(key, kind="stable")
    counts = np.bincount(key, minlength=NCORES * NCH).reshape(NCORES, NCH)
    T = np.maximum(1, -(-counts // TILE)).max(axis=0)        # tiles per chunk
    T[-1] += (-T.sum()) % (ET // TILE)                       # total multiple of 4
    total_tiles = int(T.sum())
    E_pad = total_tiles * TILE
    n_groups = E_pad // ET
    off = np.concatenate([[0], np.cumsum(T)])
    key_starts = np.concatenate([[0], np.cumsum(counts.reshape(-1))])

    eids_all = np.full((NCORES, E_pad), -1, np.int64)
    for k in range(NCORES):
        for c in range(NCH):
            cnt = counts[k, c]
            ids = order[key_starts[k * NCH + c]: key_starts[k * NCH + c] + cnt]
            eids_all[k, off[c] * TILE: off[c] * TILE + cnt] = ids

    nc = _build(T)

    nfT = np.zeros((TILE, NF_PAD), BF16)
    nfT[:, :N_NODES] = node_feat.T.astype(BF16)
    ident_t = np.eye(TILE, dtype=F32).astype(BF16)
    node_cnt = np.bincount(dst, minlength=N_NODES).astype(F32)
    invc_full = (1.0 / np.maximum(node_cnt, 1.0)).reshape(NCORES, NCH, CH)

    wmats = {"w1a": W1e[:D], "w1b": W1e[D:2 * D], "w1c": W1e[2 * D:],
             "w2e": W2e, "w1na": W1n[:D], "w1nb": W1n[D:], "w2n": W2n}
    b1e_adj = b1e - b2e @ W1e[:D]          # b2 pre-added to edges passes through W1a
    bvecs = {"b1e": b1e_adj, "b1n": b1n, "b2n": b2n}

    def wrap16(vals):
        # index i -> [i % 16, i // 16], replicated across the 8 Q7 groups
        return np.ascontiguousarray(
            np.tile(vals.reshape(-1, 16).T, (8, 1))).astype(np.int16)

    nrange = np.arange(CH, dtype=np.int64)

    in_maps = []
    for k in range(NCORES):
        eids = eids_all[k]
        valid = eids >= 0
        ev = np.zeros((E_pad, D), F32)
        ev[valid] = edge_feat[eids[valid]] + b2e[None, :]
        sw = np.zeros(E_pad, np.int64)
        sw[valid] = src[eids[valid]]
        dl = np.full(E_pad, CH + 1, np.int64)     # padding never matches 0..124
        dl[valid] = lid[eids[valid]]

        # one-hot selection tiles: sel [e,n] and its transpose selT [n,e]
        dlt = dl.reshape(total_tiles, TILE)                     # [t, e]
        sel_t = (dlt[:, :, None] == nrange[None, None, :])      # [t, e, n]
        selT_t = np.zeros((total_tiles, TILE, TILE), np.bool_)  # [t, n(128), e]
        selT_t[:, :CH, :] = sel_t.transpose(0, 2, 1)
        sel_g = sel_t.reshape(n_groups, 4, TILE, CH).transpose(0, 2, 1, 3).reshape(
            n_groups, TILE, 4 * CH)
        selT_g = selT_t.reshape(n_groups, 4, TILE, TILE).transpose(0, 2, 1, 3).reshape(
            n_groups, TILE, 4 * TILE)
        ed_g = ev.T.astype(BF16).reshape(TILE, n_groups, ET).transpose(1, 0, 2)
        stream = np.concatenate([ed_g, sel_g.astype(BF16), selT_g.astype(BF16)],
                                axis=2)                          # [g, 128, SW]
        stream = np.ascontiguousarray(
            stream.transpose(1, 0, 2).reshape(TILE, n_groups * SW))

        invc_t = np.zeros((TILE, NCH), F32)
        invc_t[:CH] = invc_full[k].T
        in_map = {
            "stream_all": stream,
            "src_w": wrap16(sw.astype(np.int16)),
            "ident": ident_t,
            "inv_cnt": invc_t,
            "nft_full": nfT,
            "nft_own": np.ascontiguousarray(node_feat[k * NPC:(k + 1) * NPC].T.astype(BF16)),
            **{n: np.ascontiguousarray(m.astype(BF16)) for n, m in wmats.items()},
            **{n: np.ascontiguousarray(v.reshape(D, 1).astype(F32)) for n, v in bvecs.items()},
        }
        in_maps.append(in_map)

    LAST_RESULTS = run_bass_kernel_spmd(nc, in_maps, core_ids=list(range(NCORES)))
    results = LAST_RESULTS.results

    node_out = np.concatenate(
        [results[k]["node_out"].astype(F32).T for k in range(NCORES)], axis=0)
    edge_out = np.empty((N_EDGES, D), F32)
    for k in range(NCORES):
        eids = eids_all[k]
        valid = eids >= 0
        eo = results[k]["edge_out"].astype(F32).T
        edge_out[eids[valid]] = eo[valid]
    return (node_out, edge_out)


# revision 8
# speedup vs baseline: 1.6190x; 1.0641x over previous
"""ALIGNN layer (edge MLP + mean-aggregation + node MLP) on 8 TRN2 NeuronCores.

Sharding: edges are partitioned by destination-node ownership — core k owns
nodes [k*2500, (k+1)*2500) and receives every edge whose dst falls in its
range, grouped by 125-node chunk so the per-node segment-sum is computed
locally with one-hot selection matmuls (no collectives needed).

Math decomposition (per edge e with endpoints s=src[e], d=dst[e]):
    z[e]  = edge[e] @ W1e[:128] + P1[s] + P2[d]        (+ b1e folded into silu bias)
    h[e]  = silu(z[e]);  edge_out[e] = edge[e] + b2e + h[e] @ W2e
where P1 = node_feat @ W1e[128:256], P2 = node_feat @ W1e[256:384] are
per-node projection tables computed on-device once. P1[src] is gathered per
edge (dma_gather, 4 SWDGE queues); P2[dst] is reconstructed without any
gather as P2_chunk.T @ one-hot(dst) since dst is chunk-local by construction.

All activations live feature-on-partition ("transposed") so the per-feature
biases become per-partition ACT biases; host supplies edge features / node
features pre-transposed plus the one-hot selection tiles, and un-transposes
the outputs.
"""
import os
import sys
import types
import numpy as np
import ml_dtypes
from contextlib import ExitStack


def _install_ntff_hook_shim():
    """This image's `antenv` lacks `axon_hooks`, so bass_utils' trace=True path
    (BASS_TRACE=1) crashes on import. Provide the two-function shim and register
    the ctypes-based NTFF hook from trn_boot so neuron-profile works."""
    try:
        import antenv
        if "antenv.axon_hooks" not in sys.modules:
            mod = types.ModuleType("antenv.axon_hooks")
            _h = [None]
            mod.set_axon_ntff_profile_hook = lambda h: _h.__setitem__(0, h)
            mod.get_axon_ntff_profile_hook = lambda: _h[0]
            sys.modules["antenv.axon_hooks"] = mod
            antenv.axon_hooks = mod
        from trn_agent_boot import trn_boot
        hook = trn_boot._ntff_profile_via_ctypes("/opt/axon/libaxon_pjrt.so")
        if hook is not None:
            sys.modules["antenv.axon_hooks"].set_axon_ntff_profile_hook(hook)
    except Exception:
        pass


_install_ntff_hook_shim()

from concourse import bacc, mybir
from concourse.tile import TileContext
from concourse.bass_utils import run_bass_kernel_spmd

BF16 = ml_dtypes.bfloat16
F32 = np.float32
dt = mybir.dt
AF = mybir.ActivationFunctionType
ALU = mybir.AluOpType

N_NODES = 20000
N_EDGES = 640000
D = 128
NCORES = 8
NPC = N_NODES // NCORES        # 2500 nodes per core
CH = 125                       # nodes per chunk
NCH = NPC // CH                # 20 chunks per core
TILE = 128
ET = 512                       # edges per pipeline group (4 tiles)
GW = 4 * CH + 4 * TILE         # sel+selT columns per group (500 + 512)
SW = ET + GW                   # full per-group stream width (edge + sel + selT)
NF_PAD = 20480                 # node table rows padded to 160*128

LAST_RESULTS = None            # BassKernelResults of the most recent run


def _build(T):
    """Build the SPMD Bass program. T[c] = tiles per chunk (same all cores)."""
    total_tiles = int(np.sum(T))
    E_pad = total_tiles * TILE
    n_groups = E_pad // ET
    off = np.concatenate([[0], np.cumsum(T)])
    tile_chunk = np.repeat(np.arange(NCH), T)
    first = off[:-1]
    last = off[1:] - 1

    nc = bacc.Bacc("TRN2", target_bir_lowering=False, debug=False,
                   num_swdge_queues=4)

    p_stream = nc.declare_dram_parameter("stream_all", [TILE, n_groups * SW], dt.bfloat16, False)
    p_srcw = nc.declare_dram_parameter("src_w", [TILE, E_pad // 16], dt.int16, False)
    p_ident = nc.declare_dram_parameter("ident", [TILE, TILE], dt.bfloat16, False)
    p_invc = nc.declare_dram_parameter("inv_cnt", [TILE, NCH], dt.float32, False)
    p_nft = nc.declare_dram_parameter("nft_full", [TILE, NF_PAD], dt.bfloat16, False)
    p_nfto = nc.declare_dram_parameter("nft_own", [TILE, NPC], dt.bfloat16, False)
    wnames = ["w1a", "w1b", "w1c", "w2e", "w1na", "w1nb", "w2n"]
    p_w = {n: nc.declare_dram_parameter(n, [TILE, TILE], dt.bfloat16, False) for n in wnames}
    bnames = ["b1e", "b1n", "b2n"]
    p_b = {n: nc.declare_dram_parameter(n, [TILE, 1], dt.float32, False) for n in bnames}
    out_e = nc.declare_dram_parameter("edge_out", [TILE, E_pad], dt.bfloat16, True)
    out_n = nc.declare_dram_parameter("node_out", [TILE, NPC], dt.bfloat16, True)

    P1d = nc.dram_tensor("p1d", [NF_PAD, D], dt.bfloat16)
    P2o = nc.dram_tensor("p2o", [NPC, D], dt.bfloat16)

    with TileContext(nc) as tc, ExitStack() as ctx:
        cp = ctx.enter_context(tc.tile_pool(name="const", bufs=1))
        wp = ctx.enter_context(tc.tile_pool(name="work", bufs=3))
        pp = ctx.enter_context(tc.tile_pool(name="psum", bufs=2, space="PSUM"))

        def cload(name, param, shape, dtype):
            t = cp.tile(shape, dtype, name=name)
            nc.sync.dma_start(out=t[:], in_=param[:])
            return t

        ident = cload("identc", p_ident, [TILE, TILE], dt.bfloat16)
        srcw = cload("srcwc", p_srcw, [TILE, E_pad // 16], dt.int16)
        invc = cload("invcc", p_invc, [TILE, NCH], dt.float32)
        nfto = cload("nftoc", p_nfto, [TILE, NPC], dt.bfloat16)
        w = {n: cload(n + "c", p_w[n], [TILE, TILE], dt.bfloat16) for n in wnames}
        b = {n: cload(n + "c", p_b[n], [TILE, 1], dt.float32) for n in bnames}
        aggbar = cp.tile([TILE, NCH * TILE], dt.bfloat16, name="aggbar")
        nout = cp.tile([TILE, NPC], dt.bfloat16, name="nout")

        # ---- Phase 0: node projection tables P1 = nf @ W1e[128:256], P2 = nf @ W1e[256:384]
        for blk in range(NF_PAD // ET):
            nft = wp.tile([TILE, ET], dt.bfloat16, tag="nft", bufs=3, name=f"nft{blk}")
            nc.sync.dma_start(out=nft[:], in_=p_nft[:, blk * ET:(blk + 1) * ET])
            ps = pp.tile([TILE, ET], dt.float32, tag="hps", bufs=2, name=f"ps0_{blk}")
            for j in range(4):
                nc.tensor.matmul(out=ps[:, j * TILE:(j + 1) * TILE],
                                 lhsT=nft[:, j * TILE:(j + 1) * TILE],
                                 rhs=w["w1b"][:], start=True, stop=True)
            st = wp.tile([TILE, ET], dt.bfloat16, tag="st1", bufs=2, name=f"st1_{blk}")
            if blk % 2 == 0:
                nc.vector.tensor_copy(out=st[:], in_=ps[:])
            else:
                nc.scalar.copy(out=st[:], in_=ps[:])
            dst_ap = P1d[blk * ET:(blk + 1) * ET, :].rearrange("(j p) h -> p j h", p=TILE)
            nc.sync.dma_start(out=dst_ap, in_=st[:].rearrange("p (j h) -> p j h", j=4))

        # ---- Phase 0b: per-core own-node projections P2o = nft_own @ W1e[256:384]
        for c in range(NCH):
            ps2 = pp.tile([CH, TILE], dt.float32, tag="agg", bufs=2, name=f"ps2_{c}")
            nc.tensor.matmul(out=ps2[:], lhsT=nfto[:, c * CH:(c + 1) * CH],
                             rhs=w["w1c"][:], start=True, stop=True)
            st2 = wp.tile([CH, TILE], dt.bfloat16, tag="st2", bufs=2, name=f"st2_{c}")
            if c % 2 == 0:
                nc.vector.tensor_copy(out=st2[:], in_=ps2[:])
            else:
                nc.scalar.copy(out=st2[:], in_=ps2[:])
            nc.sync.dma_start(out=P2o[c * CH:(c + 1) * CH, :], in_=st2[:])

        # ---- Phase 1: edge pipeline
        agg_tiles = [None] * NCH
        p2c_tiles = [None] * NCH
        for g in range(n_groups):
            sl_ = slice(g * ET, (g + 1) * ET)
            stg = wp.tile([TILE, SW], dt.bfloat16, tag="stg", bufs=4, name=f"stg{g}")
            nc.sync.dma_start(out=stg[:], in_=p_stream[:, g * SW:(g + 1) * SW])
            ed = stg[:, :ET]
            selg = stg[:, ET:]
            s1 = wp.tile([TILE, ET], dt.bfloat16, tag="s1", bufs=6, name=f"s1_{g}")
            nc.gpsimd.dma_gather(
                out_ap=s1[:].rearrange("p (o e) -> p o e", o=1), in_ap=P1d[:],
                idxs_ap=srcw[:, g * 32:(g + 1) * 32], num_idxs=ET, num_idxs_reg=ET,
                elem_size=D, transpose=True, queue_num=(g % 4))

            # chunk-contiguous segments inside the group, and P2 chunk row loads
            segs = []
            for j in range(4):
                c = int(tile_chunk[4 * g + j])
                if segs and segs[-1][0] == c:
                    segs[-1][2] = j
                else:
                    segs.append([c, j, j])
                if 4 * g + j == first[c]:
                    p2c_tiles[c] = wp.tile([CH, TILE], dt.bfloat16, tag="p2c", bufs=2,
                                           name=f"p2c{c}")
                    nc.scalar.dma_start(out=p2c_tiles[c][:],
                                        in_=P2o[c * CH:(c + 1) * CH, :])

            hps = pp.tile([TILE, ET], dt.float32, tag="hps", bufs=2, name=f"hps{g}")
            nc.tensor.matmul(out=hps[:], lhsT=w["w1a"][:], rhs=ed, start=True, stop=False)
            nc.tensor.matmul(out=hps[:], lhsT=ident[:], rhs=s1[:], start=False, stop=False)
            for i, (c, j0, j1) in enumerate(segs):
                nc.tensor.matmul(
                    out=hps[:, j0 * TILE:(j1 + 1) * TILE],
                    lhsT=p2c_tiles[c][:],
                    rhs=selg[:CH, 4 * CH + j0 * TILE: 4 * CH + (j1 + 1) * TILE],
                    start=False, stop=(i == len(segs) - 1), skip_group_check=True)

            ht = wp.tile([TILE, ET], dt.bfloat16, tag="ht", bufs=4, name=f"ht{g}")
            nc.scalar.activation(out=ht[:], in_=hps[:], func=AF.Silu, bias=b["b1e"][:, :1])

            dps = pp.tile([TILE, ET], dt.float32, tag="dps", bufs=2, name=f"dps{g}")
            nc.tensor.matmul(out=dps[:], lhsT=w["w2e"][:], rhs=ht[:], start=True, stop=True)
            eo = wp.tile([TILE, ET], dt.bfloat16, tag="eo", bufs=4, name=f"eo{g}")
            nc.vector.tensor_tensor(out=eo[:], in0=dps[:], in1=ed, op=ALU.add)
            nc.sync.dma_start(out=out_e[:, sl_], in_=eo[:])

            for j in range(4):
                t = 4 * g + j
                c = int(tile_chunk[t])
                tp = pp.tile([TILE, TILE], dt.bfloat16, tag="tp", bufs=2, name=f"tp{t}")
                nc.tensor.transpose(out=tp[:], in_=eo[:, j * TILE:(j + 1) * TILE],
                                    identity=ident[:])
                rt = wp.tile([TILE, TILE], dt.bfloat16, tag="rt", bufs=4, name=f"rt{t}")
                nc.vector.tensor_copy(out=rt[:], in_=tp[:])
                if t == first[c]:
                    agg_tiles[c] = pp.tile([CH, TILE], dt.float32, tag="agg", bufs=2,
                                           name=f"agg{c}")
                nc.tensor.matmul(out=agg_tiles[c][:],
                                 lhsT=selg[:, j * CH:(j + 1) * CH], rhs=rt[:],
                                 start=(t == first[c]), stop=(t == last[c]),
                                 skip_group_check=True)
                if t == last[c]:
                    nc.vector.tensor_scalar(out=aggbar[:CH, c * TILE:(c + 1) * TILE],
                                            in0=agg_tiles[c][:],
                                            scalar1=invc[:CH, c:c + 1], scalar2=None,
                                            op0=ALU.mult)

        # ---- Phase 2: node MLP per chunk
        for c in range(NCH):
            tp2 = pp.tile([TILE, TILE], dt.bfloat16, tag="tp", bufs=2, name=f"tp2_{c}")
            nc.tensor.transpose(out=tp2[:, :CH], in_=aggbar[:CH, c * TILE:(c + 1) * TILE],
                                identity=ident[:CH, :CH])
            abt = wp.tile([TILE, CH], dt.bfloat16, tag="abt", bufs=2, name=f"abt{c}")
            nc.vector.tensor_copy(out=abt[:], in_=tp2[:, :CH])
            nsl = slice(c * CH, (c + 1) * CH)
            gps = pp.tile([TILE, CH], dt.float32, tag="agg", bufs=2, name=f"gps{c}")
            nc.tensor.matmul(out=gps[:], lhsT=w["w1na"][:], rhs=nfto[:, nsl],
                             start=True, stop=False)
            nc.tensor.matmul(out=gps[:], lhsT=w["w1nb"][:], rhs=abt[:],
                             start=False, stop=True)
            gt = wp.tile([TILE, CH], dt.bfloat16, tag="gt", bufs=2, name=f"gt{c}")
            nc.scalar.activation(out=gt[:], in_=gps[:], func=AF.Silu, bias=b["b1n"][:, :1])
            ops = pp.tile([TILE, CH], dt.float32, tag="tp", bufs=2, name=f"ops{c}")
            nc.tensor.matmul(out=ops[:], lhsT=w["w2n"][:], rhs=gt[:], start=True, stop=True)
            t3 = wp.tile([TILE, CH], dt.bfloat16, tag="t3", bufs=2, name=f"t3_{c}")
            nc.scalar.activation(out=t3[:], in_=ops[:], func=AF.Identity, bias=b["b2n"][:, :1])
            nc.vector.tensor_tensor(out=nout[:, nsl], in0=t3[:], in1=nfto[:, nsl], op=ALU.add)
        nc.sync.dma_start(out=out_n[:], in_=nout[:])

    nc.compile()
    return nc


def kernel(**inputs):
    global LAST_RESULTS
    node_feat = np.ascontiguousarray(np.asarray(inputs["node_feat"], F32))
    edge_feat = np.ascontiguousarray(np.asarray(inputs["edge_feat"], F32))
    edge_index = np.asarray(inputs["edge_index"])
    W1e = np.asarray(inputs["W1e"], F32)
    b1e = np.asarray(inputs["b1e"], F32)
    W2e = np.asarray(inputs["W2e"], F32)
    b2e = np.asarray(inputs["b2e"], F32)
    W1n = np.asarray(inputs["W1n"], F32)
    b1n = np.asarray(inputs["b1n"], F32)
    W2n = np.asarray(inputs["W2n"], F32)
    b2n = np.asarray(inputs["b2n"], F32)

    src = edge_index[0].astype(np.int64)
    dst = edge_index[1].astype(np.int64)

    # ---- host-side distribution: bucket edges by (owner core, 125-node chunk)
    core = dst // NPC
    loc = dst - core * NPC
    chunk = loc // CH
    lid = (loc - chunk * CH).astype(np.int64)
    key = core * NCH + chunk
    order = np.argsort(key, kind="stable")
    counts = np.bincount(key, minlength=NCORES * NCH).reshape(NCORES, NCH)
    T = np.maximum(1, -(-counts // TILE)).max(axis=0)        # tiles per chunk
    T[-1] += (-T.sum()) % (ET // TILE)                       # total multiple of 4
    total_tiles = int(T.sum())
    E_pad = total_tiles * TILE
    n_groups = E_pad // ET
    off = np.concatenate([[0], np.cumsum(T)])
    key_starts = np.concatenate([[0], np.cumsum(counts.reshape(-1))])

    eids_all = np.full((NCORES, E_pad), -1, np.int64)
    for k in range(NCORES):
        for c in range(NCH):
            cnt = counts[k, c]
            ids = order[key_starts[k * NCH + c]: key_starts[k * NCH + c] + cnt]
            eids_all[k, off[c] * TILE: off[c] * TILE + cnt] = ids

    nc = _build(T)

    nfT = np.zeros((TILE, NF_PAD), BF16)
    nfT[:, :N_NODES] = node_feat.T.astype(BF16)
    ident_t = np.eye(TILE, dtype=F32).astype(BF16)
    node_cnt = np.bincount(dst, minlength=N_NODES).astype(F32)
    invc_full = (1.0 / np.maximum(node_cnt, 1.0)).reshape(NCORES, NCH, CH)

    wmats = {"w1a": W1e[:D], "w1b": W1e[D:2 * D], "w1c": W1e[2 * D:],
             "w2e": W2e, "w1na": W1n[:D], "w1nb": W1n[D:], "w2n": W2n}
    b1e_adj = b1e - b2e @ W1e[:D]          # b2 pre-added to edges passes through W1a
    bvecs = {"b1e": b1e_adj, "b1n": b1n, "b2n": b2n}

    def wrap16(vals):
        # index i -> [i % 16, i // 16], replicated across the 8 Q7 groups
        return np.ascontiguousarray(
            np.tile(vals.reshape(-1, 16).T, (8, 1))).astype(np.int16)

    nrange = np.arange(CH, dtype=np.int64)

    in_maps = []
    for k in range(NCORES):
        eids = eids_all[k]
        valid = eids >= 0
        ev = np.zeros((E_pad, D), F32)
        ev[valid] = edge_feat[eids[valid]] + b2e[None, :]
        sw = np.zeros(E_pad, np.int64)
        sw[valid] = src[eids[valid]]
        dl = np.full(E_pad, CH + 1, np.int64)     # padding never matches 0..124
        dl[valid] = lid[eids[valid]]

        # one-hot selection tiles: sel [e,n] and its transpose selT [n,e]
        dlt = dl.reshape(total_tiles, TILE)                     # [t, e]
        sel_t = (dlt[:, :, None] == nrange[None, None, :])      # [t, e, n]
        selT_t = np.zeros((total_tiles, TILE, TILE), np.bool_)  # [t, n(128), e]
        selT_t[:, :CH, :] = sel_t.transpose(0, 2, 1)
        sel_g = sel_t.reshape(n_groups, 4, TILE, CH).transpose(0, 2, 1, 3).reshape(
            n_groups, TILE, 4 * CH)
        selT_g = selT_t.reshape(n_groups, 4, TILE, TILE).transpose(0, 2, 1, 3).reshape(
            n_groups, TILE, 4 * TILE)
        ed_g = ev.T.astype(BF16).reshape(TILE, n_groups, ET).transpose(1, 0, 2)
        stream = np.concatenate([ed_g, sel_g.astype(BF16), selT_g.astype(BF16)],
                                axis=2)                          # [g, 128, SW]
        stream = np.ascontiguousarray(
            stream.transpose(1, 0, 2).reshape(TILE, n_groups * SW))

        invc_t = np.zeros((TILE, NCH), F32)
        invc_t[:CH] = invc_full[k].T
        in_map = {
            "stream_all": stream,
            "src_w": wrap16(sw.astype(np.int16)),
            "ident": ident_t,
            "inv_cnt": invc_t,
            "nft_full": nfT,
            "nft_own": np.ascontiguousarray(node_feat[k * NPC:(k + 1) * NPC].T.astype(BF16)),
            **{n: np.ascontiguousarray(m.astype(BF16)) for n, m in wmats.items()},
            **{n: np.ascontiguousarray(v.reshape(D, 1).astype(F32)) for n, v in bvecs.items()},
        }
        in_maps.append(in_map)

    LAST_RESULTS = run_bass_kernel_spmd(nc, in_maps, core_ids=list(range(NCORES)))
    results = LAST_RESULTS.results

    node_out = np.concatenate(
        [results[k]["node_out"].astype(F32).T for k in range(NCORES)], axis=0)
    edge_out = np.empty((N_EDGES, D), F32)
    for k in range(NCORES):
        eids = eids_all[k]
        valid = eids >= 0
        eo = results[k]["edge_out"].astype(F32).T
        edge_out[eids[valid]] = eo[valid]
    return (node_out, edge_out)


# revision 9
# speedup vs baseline: 1.7224x; 1.0638x over previous
"""ALIGNN layer (edge MLP + mean-aggregation + node MLP) on 8 TRN2 NeuronCores.

Sharding: edges are partitioned by destination-node ownership — core k owns
nodes [k*2500, (k+1)*2500) and receives every edge whose dst falls in its
range, grouped by 125-node chunk so the per-node segment-sum is computed
locally with one-hot selection matmuls (no collectives needed).

Math decomposition (per edge e with endpoints s=src[e], d=dst[e]):
    z[e]  = edge[e] @ W1e[:128] + P1[s] + P2[d]        (+ b1e folded into silu bias)
    h[e]  = silu(z[e]);  edge_out[e] = edge[e] + b2e + h[e] @ W2e
where P1 = node_feat @ W1e[128:256], P2 = node_feat @ W1e[256:384] are
per-node projection tables computed on-device once. P1[src] is gathered per
edge (dma_gather, 4 SWDGE queues); P2[dst] is reconstructed without any
gather as P2_chunk.T @ one-hot(dst) since dst is chunk-local by construction.

All activations live feature-on-partition ("transposed") so the per-feature
biases become per-partition ACT biases; host supplies edge features / node
features pre-transposed plus the one-hot selection tiles, and un-transposes
the outputs.
"""
import os
import sys
import types
import numpy as np
import ml_dtypes
from contextlib import ExitStack


def _install_ntff_hook_shim():
    """This image's `antenv` lacks `axon_hooks`, so bass_utils' trace=True path
    (BASS_TRACE=1) crashes on import. Provide the two-function shim and register
    the ctypes-based NTFF hook from trn_boot so neuron-profile works."""
    try:
        import antenv
        if "antenv.axon_hooks" not in sys.modules:
            mod = types.ModuleType("antenv.axon_hooks")
            _h = [None]
            mod.set_axon_ntff_profile_hook = lambda h: _h.__setitem__(0, h)
            mod.get_axon_ntff_profile_hook = lambda: _h[0]
            sys.modules["antenv.axon_hooks"] = mod
            antenv.axon_hooks = mod
        from trn_agent_boot import trn_boot
        hook = trn_boot._ntff_profile_via_ctypes("/opt/axon/libaxon_pjrt.so")
        if hook is not None:
            sys.modules["antenv.axon_hooks"].set_axon_ntff_profile_hook(hook)
    except Exception:
        pass


_install_ntff_hook_shim()

from concourse import bacc, mybir
from concourse.tile import TileContext
from concourse.bass_utils import run_bass_kernel_spmd

BF16 = ml_dtypes.bfloat16
F32 = np.float32
dt = mybir.dt
AF = mybir.ActivationFunctionType
ALU = mybir.AluOpType

N_NODES = 20000
N_EDGES = 640000
D = 128
NCORES = 8
NPC = N_NODES // NCORES        # 2500 nodes per core
CH = 125                       # nodes per chunk
NCH = NPC // CH                # 20 chunks per core
TILE = 128
ET = 512                       # edges per pipeline group (4 tiles)
GW = 4 * CH + 4 * TILE         # sel+selT columns per group (500 + 512)
SW = ET + 4 * TILE             # per-group stream width (edge + selT)
NF_PAD = 20480                 # node table rows padded to 160*128

LAST_RESULTS = None            # BassKernelResults of the most recent run


def _build(T):
    """Build the SPMD Bass program. T[c] = tiles per chunk (same all cores)."""
    total_tiles = int(np.sum(T))
    E_pad = total_tiles * TILE
    n_groups = E_pad // ET
    off = np.concatenate([[0], np.cumsum(T)])
    tile_chunk = np.repeat(np.arange(NCH), T)
    first = off[:-1]
    last = off[1:] - 1

    nc = bacc.Bacc("TRN2", target_bir_lowering=False, debug=False,
                   num_swdge_queues=4)

    p_stream = nc.declare_dram_parameter("stream_all", [TILE, n_groups * SW], dt.bfloat16, False)
    p_dstloc = nc.declare_dram_parameter("dst_loc", [TILE, total_tiles], dt.bfloat16, False)
    p_iota4 = nc.declare_dram_parameter("iota4", [TILE, 4 * CH], dt.bfloat16, False)
    p_srcw = nc.declare_dram_parameter("src_w", [TILE, E_pad // 16], dt.int16, False)
    p_ident = nc.declare_dram_parameter("ident", [TILE, TILE], dt.bfloat16, False)
    p_invc = nc.declare_dram_parameter("inv_cnt", [TILE, NCH], dt.float32, False)
    p_nft = nc.declare_dram_parameter("nft_full", [TILE, NF_PAD], dt.bfloat16, False)
    p_nfto = nc.declare_dram_parameter("nft_own", [TILE, NPC], dt.bfloat16, False)
    wnames = ["w1a", "w1b", "w1c", "w2e", "w1na", "w1nb", "w2n"]
    p_w = {n: nc.declare_dram_parameter(n, [TILE, TILE], dt.bfloat16, False) for n in wnames}
    bnames = ["b1e", "b1n", "b2n"]
    p_b = {n: nc.declare_dram_parameter(n, [TILE, 1], dt.float32, False) for n in bnames}
    out_e = nc.declare_dram_parameter("edge_out", [TILE, E_pad], dt.bfloat16, True)
    out_n = nc.declare_dram_parameter("node_out", [TILE, NPC], dt.bfloat16, True)

    P1d = nc.dram_tensor("p1d", [NF_PAD, D], dt.bfloat16)
    P2o = nc.dram_tensor("p2o", [NPC, D], dt.bfloat16)

    with TileContext(nc) as tc, ExitStack() as ctx:
        cp = ctx.enter_context(tc.tile_pool(name="const", bufs=1))
        wp = ctx.enter_context(tc.tile_pool(name="work", bufs=3))
        pp = ctx.enter_context(tc.tile_pool(name="psum", bufs=2, space="PSUM"))

        def cload(name, param, shape, dtype):
            t = cp.tile(shape, dtype, name=name)
            nc.sync.dma_start(out=t[:], in_=param[:])
            return t

        ident = cload("identc", p_ident, [TILE, TILE], dt.bfloat16)
        dstloc = cload("dstlocc", p_dstloc, [TILE, total_tiles], dt.bfloat16)
        iota4 = cload("iota4c", p_iota4, [TILE, 4 * CH], dt.bfloat16)
        srcw = cload("srcwc", p_srcw, [TILE, E_pad // 16], dt.int16)
        invc = cload("invcc", p_invc, [TILE, NCH], dt.float32)
        nfto = cload("nftoc", p_nfto, [TILE, NPC], dt.bfloat16)
        w = {n: cload(n + "c", p_w[n], [TILE, TILE], dt.bfloat16) for n in wnames}
        b = {n: cload(n + "c", p_b[n], [TILE, 1], dt.float32) for n in bnames}
        aggbar = cp.tile([TILE, NCH * TILE], dt.bfloat16, name="aggbar")
        nout = cp.tile([TILE, NPC], dt.bfloat16, name="nout")

        # ---- Phase 0: node projection tables P1 = nf @ W1e[128:256], P2 = nf @ W1e[256:384]
        for blk in range(NF_PAD // ET):
            nft = wp.tile([TILE, ET], dt.bfloat16, tag="nft", bufs=3, name=f"nft{blk}")
            nc.sync.dma_start(out=nft[:], in_=p_nft[:, blk * ET:(blk + 1) * ET])
            ps = pp.tile([TILE, ET], dt.float32, tag="hps", bufs=2, name=f"ps0_{blk}")
            for j in range(4):
                nc.tensor.matmul(out=ps[:, j * TILE:(j + 1) * TILE],
                                 lhsT=nft[:, j * TILE:(j + 1) * TILE],
                                 rhs=w["w1b"][:], start=True, stop=True)
            st = wp.tile([TILE, ET], dt.bfloat16, tag="st1", bufs=2, name=f"st1_{blk}")
            if blk % 2 == 0:
                nc.vector.tensor_copy(out=st[:], in_=ps[:])
            else:
                nc.scalar.copy(out=st[:], in_=ps[:])
            dst_ap = P1d[blk * ET:(blk + 1) * ET, :].rearrange("(j p) h -> p j h", p=TILE)
            nc.sync.dma_start(out=dst_ap, in_=st[:].rearrange("p (j h) -> p j h", j=4))

        # ---- Phase 0b: per-core own-node projections P2o = nft_own @ W1e[256:384]
        for c in range(NCH):
            ps2 = pp.tile([CH, TILE], dt.float32, tag="agg", bufs=2, name=f"ps2_{c}")
            nc.tensor.matmul(out=ps2[:], lhsT=nfto[:, c * CH:(c + 1) * CH],
                             rhs=w["w1c"][:], start=True, stop=True)
            st2 = wp.tile([CH, TILE], dt.bfloat16, tag="st2", bufs=2, name=f"st2_{c}")
            if c % 2 == 0:
                nc.vector.tensor_copy(out=st2[:], in_=ps2[:])
            else:
                nc.scalar.copy(out=st2[:], in_=ps2[:])
            nc.sync.dma_start(out=P2o[c * CH:(c + 1) * CH, :], in_=st2[:])

        # ---- Phase 1: edge pipeline
        agg_tiles = [None] * NCH
        p2c_tiles = [None] * NCH
        for g in range(n_groups):
            sl_ = slice(g * ET, (g + 1) * ET)
            stg = wp.tile([TILE, SW], dt.bfloat16, tag="stg", bufs=6, name=f"stg{g}")
            nc.sync.dma_start(out=stg[:], in_=p_stream[:, g * SW:(g + 1) * SW])
            ed = stg[:, :ET]
            selg = stg[:, ET:]
            selb = wp.tile([TILE, 4 * CH], dt.bfloat16, tag="selb", bufs=4, name=f"selb{g}")
            nc.vector.tensor_tensor(
                out=selb[:].rearrange("p (j n) -> p j n", j=4),
                in0=iota4[:].rearrange("p (j n) -> p j n", j=4),
                in1=dstloc[:, 4 * g:4 * g + 4].to_broadcast([TILE, 4, CH]),
                op=ALU.is_equal)
            s1 = wp.tile([TILE, ET], dt.bfloat16, tag="s1", bufs=10, name=f"s1_{g}")
            nc.gpsimd.dma_gather(
                out_ap=s1[:].rearrange("p (o e) -> p o e", o=1), in_ap=P1d[:],
                idxs_ap=srcw[:, g * 32:(g + 1) * 32], num_idxs=ET, num_idxs_reg=ET,
                elem_size=D, transpose=True, queue_num=(g % 4))

            # chunk-contiguous segments inside the group, and P2 chunk row loads
            segs = []
            for j in range(4):
                c = int(tile_chunk[4 * g + j])
                if segs and segs[-1][0] == c:
                    segs[-1][2] = j
                else:
                    segs.append([c, j, j])
                if 4 * g + j == first[c]:
                    p2c_tiles[c] = wp.tile([CH, TILE], dt.bfloat16, tag="p2c", bufs=2,
                                           name=f"p2c{c}")
                    nc.scalar.dma_start(out=p2c_tiles[c][:],
                                        in_=P2o[c * CH:(c + 1) * CH, :])

            hps = pp.tile([TILE, ET], dt.float32, tag="hps", bufs=2, name=f"hps{g}")
            nc.tensor.matmul(out=hps[:], lhsT=w["w1a"][:], rhs=ed, start=True, stop=False)
            nc.tensor.matmul(out=hps[:], lhsT=ident[:], rhs=s1[:], start=False, stop=False)
            for i, (c, j0, j1) in enumerate(segs):
                nc.tensor.matmul(
                    out=hps[:, j0 * TILE:(j1 + 1) * TILE],
                    lhsT=p2c_tiles[c][:],
                    rhs=selg[:CH, j0 * TILE:(j1 + 1) * TILE],
                    start=False, stop=(i == len(segs) - 1), skip_group_check=True)

            ht = wp.tile([TILE, ET], dt.bfloat16, tag="ht", bufs=4, name=f"ht{g}")
            nc.scalar.activation(out=ht[:], in_=hps[:], func=AF.Silu, bias=b["b1e"][:, :1])

            dps = pp.tile([TILE, ET], dt.float32, tag="dps", bufs=2, name=f"dps{g}")
            nc.tensor.matmul(out=dps[:], lhsT=w["w2e"][:], rhs=ht[:], start=True, stop=True)
            eo = wp.tile([TILE, ET], dt.bfloat16, tag="eo", bufs=4, name=f"eo{g}")
            nc.vector.tensor_tensor(out=eo[:], in0=dps[:], in1=ed, op=ALU.add)
            nc.sync.dma_start(out=out_e[:, sl_], in_=eo[:])

            for j in range(4):
                t = 4 * g + j
                c = int(tile_chunk[t])
                tp = pp.tile([TILE, TILE], dt.bfloat16, tag="tp", bufs=2, name=f"tp{t}")
                nc.tensor.transpose(out=tp[:], in_=eo[:, j * TILE:(j + 1) * TILE],
                                    identity=ident[:])
                rt = wp.tile([TILE, TILE], dt.bfloat16, tag="rt", bufs=4, name=f"rt{t}")
                nc.vector.tensor_copy(out=rt[:], in_=tp[:])
                if t == first[c]:
                    agg_tiles[c] = pp.tile([CH, TILE], dt.float32, tag="agg", bufs=2,
                                           name=f"agg{c}")
                nc.tensor.matmul(out=agg_tiles[c][:],
                                 lhsT=selb[:, j * CH:(j + 1) * CH], rhs=rt[:],
                                 start=(t == first[c]), stop=(t == last[c]),
                                 skip_group_check=True)
                if t == last[c]:
                    nc.vector.tensor_scalar(out=aggbar[:CH, c * TILE:(c + 1) * TILE],
                                            in0=agg_tiles[c][:],
                                            scalar1=invc[:CH, c:c + 1], scalar2=None,
                                            op0=ALU.mult)

        # ---- Phase 2: node MLP per chunk
        for c in range(NCH):
            tp2 = pp.tile([TILE, TILE], dt.bfloat16, tag="tp", bufs=2, name=f"tp2_{c}")
            nc.tensor.transpose(out=tp2[:, :CH], in_=aggbar[:CH, c * TILE:(c + 1) * TILE],
                                identity=ident[:CH, :CH])
            abt = wp.tile([TILE, CH], dt.bfloat16, tag="abt", bufs=2, name=f"abt{c}")
            nc.vector.tensor_copy(out=abt[:], in_=tp2[:, :CH])
            nsl = slice(c * CH, (c + 1) * CH)
            gps = pp.tile([TILE, CH], dt.float32, tag="agg", bufs=2, name=f"gps{c}")
            nc.tensor.matmul(out=gps[:], lhsT=w["w1na"][:], rhs=nfto[:, nsl],
                             start=True, stop=False)
            nc.tensor.matmul(out=gps[:], lhsT=w["w1nb"][:], rhs=abt[:],
                             start=False, stop=True)
            gt = wp.tile([TILE, CH], dt.bfloat16, tag="gt", bufs=2, name=f"gt{c}")
            nc.scalar.activation(out=gt[:], in_=gps[:], func=AF.Silu, bias=b["b1n"][:, :1])
            ops = pp.tile([TILE, CH], dt.float32, tag="tp", bufs=2, name=f"ops{c}")
            nc.tensor.matmul(out=ops[:], lhsT=w["w2n"][:], rhs=gt[:], start=True, stop=True)
            t3 = wp.tile([TILE, CH], dt.bfloat16, tag="t3", bufs=2, name=f"t3_{c}")
            nc.scalar.activation(out=t3[:], in_=ops[:], func=AF.Identity, bias=b["b2n"][:, :1])
            nc.vector.tensor_tensor(out=nout[:, nsl], in0=t3[:], in1=nfto[:, nsl], op=ALU.add)
        nc.sync.dma_start(out=out_n[:], in_=nout[:])

    nc.compile()
    return nc


def kernel(**inputs):
    global LAST_RESULTS
    node_feat = np.ascontiguousarray(np.asarray(inputs["node_feat"], F32))
    edge_feat = np.ascontiguousarray(np.asarray(inputs["edge_feat"], F32))
    edge_index = np.asarray(inputs["edge_index"])
    W1e = np.asarray(inputs["W1e"], F32)
    b1e = np.asarray(inputs["b1e"], F32)
    W2e = np.asarray(inputs["W2e"], F32)
    b2e = np.asarray(inputs["b2e"], F32)
    W1n = np.asarray(inputs["W1n"], F32)
    b1n = np.asarray(inputs["b1n"], F32)
    W2n = np.asarray(inputs["W2n"], F32)
    b2n = np.asarray(inputs["b2n"], F32)

    src = edge_index[0].astype(np.int64)
    dst = edge_index[1].astype(np.int64)

    # ---- host-side distribution: bucket edges by (owner core, 125-node chunk)
    core = dst // NPC
    loc = dst - core * NPC
    chunk = loc // CH
    lid = (loc - chunk * CH).astype(np.int64)
    key = core * NCH + chunk
    order = np.argsort(key, kind="stable")
    counts = np.bincount(key, minlength=NCORES * NCH).reshape(NCORES, NCH)
    T = np.maximum(1, -(-counts // TILE)).max(axis=0)        # tiles per chunk
    T[-1] += (-T.sum()) % (ET // TILE)                       # total multiple of 4
    total_tiles = int(T.sum())
    E_pad = total_tiles * TILE
    n_groups = E_pad // ET
    off = np.concatenate([[0], np.cumsum(T)])
    key_starts = np.concatenate([[0], np.cumsum(counts.reshape(-1))])

    eids_all = np.full((NCORES, E_pad), -1, np.int64)
    for k in range(NCORES):
        for c in range(NCH):
            cnt = counts[k, c]
            ids = order[key_starts[k * NCH + c]: key_starts[k * NCH + c] + cnt]
            eids_all[k, off[c] * TILE: off[c] * TILE + cnt] = ids

    nc = _build(T)

    nfT = np.zeros((TILE, NF_PAD), BF16)
    nfT[:, :N_NODES] = node_feat.T.astype(BF16)
    ident_t = np.eye(TILE, dtype=F32).astype(BF16)
    node_cnt = np.bincount(dst, minlength=N_NODES).astype(F32)
    invc_full = (1.0 / np.maximum(node_cnt, 1.0)).reshape(NCORES, NCH, CH)

    wmats = {"w1a": W1e[:D], "w1b": W1e[D:2 * D], "w1c": W1e[2 * D:],
             "w2e": W2e, "w1na": W1n[:D], "w1nb": W1n[D:], "w2n": W2n}
    b1e_adj = b1e - b2e @ W1e[:D]          # b2 pre-added to edges passes through W1a
    bvecs = {"b1e": b1e_adj, "b1n": b1n, "b2n": b2n}

    def wrap16(vals):
        # index i -> [i % 16, i // 16], replicated across the 8 Q7 groups
        return np.ascontiguousarray(
            np.tile(vals.reshape(-1, 16).T, (8, 1))).astype(np.int16)

    nrange = np.arange(CH, dtype=np.int64)

    in_maps = []
    for k in range(NCORES):
        eids = eids_all[k]
        valid = eids >= 0
        ev = np.zeros((E_pad, D), F32)
        ev[valid] = edge_feat[eids[valid]] + b2e[None, :]
        sw = np.zeros(E_pad, np.int64)
        sw[valid] = src[eids[valid]]
        dl = np.full(E_pad, CH + 1, np.int64)     # padding never matches 0..124
        dl[valid] = lid[eids[valid]]

        # one-hot selT [n,e] tiles (sel is built on-device from dst_loc)
        dlt = dl.reshape(total_tiles, TILE)                     # [t, e]
        selT_t = np.zeros((total_tiles, TILE, TILE), np.bool_)  # [t, n(128), e]
        selT_t[:, :CH, :] = dlt[:, None, :] == nrange[None, :, None]
        selT_g = selT_t.reshape(n_groups, 4, TILE, TILE).transpose(0, 2, 1, 3).reshape(
            n_groups, TILE, 4 * TILE)
        ed_g = ev.T.astype(BF16).reshape(TILE, n_groups, ET).transpose(1, 0, 2)
        stream = np.concatenate([ed_g, selT_g.astype(BF16)], axis=2)  # [g, 128, SW]
        stream = np.ascontiguousarray(
            stream.transpose(1, 0, 2).reshape(TILE, n_groups * SW))

        invc_t = np.zeros((TILE, NCH), F32)
        invc_t[:CH] = invc_full[k].T
        in_map = {
            "stream_all": stream,
            "dst_loc": np.ascontiguousarray(dlt.T.astype(BF16)),
            "iota4": np.ascontiguousarray(
                np.tile(np.arange(CH, dtype=F32), (TILE, 4))).astype(BF16),
            "src_w": wrap16(sw.astype(np.int16)),
            "ident": ident_t,
            "inv_cnt": invc_t,
            "nft_full": nfT,
            "nft_own": np.ascontiguousarray(node_feat[k * NPC:(k + 1) * NPC].T.astype(BF16)),
            **{n: np.ascontiguousarray(m.astype(BF16)) for n, m in wmats.items()},
            **{n: np.ascontiguousarray(v.reshape(D, 1).astype(F32)) for n, v in bvecs.items()},
        }
        in_maps.append(in_map)

    LAST_RESULTS = run_bass_kernel_spmd(nc, in_maps, core_ids=list(range(NCORES)))
    results = LAST_RESULTS.results

    node_out = np.concatenate(
        [results[k]["node_out"].astype(F32).T for k in range(NCORES)], axis=0)
    edge_out = np.empty((N_EDGES, D), F32)
    for k in range(NCORES):
        eids = eids_all[k]
        valid = eids >= 0
        eo = results[k]["edge_out"].astype(F32).T
        edge_out[eids[valid]] = eo[valid]
    return (node_out, edge_out)
